# revision 9
# baseline (speedup 1.0000x reference)
"""AttnBlock fusion kernel for Trainium2 (Bass/Tile), 8 NeuronCores.

Reference computation (per batch element b; c=512 channels, hw=1024 spatial):
    h  = GroupNorm(32, c)(x) ; k = Wk h + bk ; v = Wv h + bv
    y_ = GroupNorm(32, c)(y) ; q = Wq y_ + bq
    attn = softmax_j(q^T k / sqrt(c)) ; o = v @ attn^T ; out = x + Wp o + bp

Sharding: pure data parallel over batch (16 batches / 8 cores = 2 each).

Key algebraic moves (exactness notes inline):
  * bk cancels in softmax over j (adds a per-i constant to logits) -> dropped.
  * bv contributes bv * sum_j(attn) = bv exactly -> folded into
    bp' = bp + Wp @ bv on the host.
  * v is produced directly transposed (vT[j,c]) by the projection matmul,
    and attention scores are computed as S[j,i]; no on-chip transposes.
  * softmax denominator: ones[128,128] matmul gives the partition-broadcast
    column sum of exp(S) directly in PSUM.
All matmuls run in float32r (full fp32 storage, 1 cycle/row at N=512).
"""

import math
import os
import sys

import numpy as np

for _p in ("/opt/trn_rl_repo", "/root/.axon_site/_ro/trn_rl_repo"):
    if os.path.isdir(_p) and _p not in sys.path:
        sys.path.append(_p)

import concourse.bass as bass
import concourse.bacc as bacc
import concourse.mybir as mybir
import concourse.tile as tile
from concourse.bass_utils import run_bass_kernel_spmd

F32 = mybir.dt.float32
F32R = mybir.dt.float32r
AF = mybir.ActivationFunctionType
ALU = mybir.AluOpType

B, C, H, W = 16, 512, 32, 32
HW = H * W                  # 1024
NCORES = 8
BPC = B // NCORES           # 2 batches per core
P = 128                     # SBUF partitions
CT = C // P                 # 4 channel tiles
JT = HW // P                # 8 key-position tiles
IBS = 512                   # i-block size (query positions per block)
IB = HW // IBS              # 2 i-blocks
GROUPS = 32
GSIZE = C // GROUPS         # 16 channels per group
EPS = 1e-6
SM_SCALE = float(int(C) ** -0.5)

# prm rows
R_GN_SCALE, R_GN_BIAS, R_GN1_SCALE, R_GN1_BIAS, R_BQ, R_BPP = range(6)


def _r(ap):
    """View an fp32 AP as float32r for the tensor engine."""
    return ap.bitcast(F32R)


def _emit(tc, aps):
    nc = tc.nc
    xs, ys, wq, wk, wv, wp, prm, amat, out = (
        aps["xs"], aps["ys"], aps["wqT"], aps["wkT"], aps["wvT"], aps["wpT"],
        aps["prm"], aps["amat"], aps["out"],
    )

    from contextlib import ExitStack

    with ExitStack() as ctx:
        cpool = ctx.enter_context(tc.tile_pool(name="const", bufs=1))
        wpool = ctx.enter_context(tc.tile_pool(name="w", bufs=1))
        xpool = ctx.enter_context(tc.tile_pool(name="xin", bufs=2))
        ypool = ctx.enter_context(tc.tile_pool(name="yin", bufs=1))
        ynpool = ctx.enter_context(tc.tile_pool(name="yn", bufs=1))
        hpool = ctx.enter_context(tc.tile_pool(name="hb", bufs=1))
        qpool = ctx.enter_context(tc.tile_pool(name="qb", bufs=1))
        kpool = ctx.enter_context(tc.tile_pool(name="kb", bufs=1))
        vpool = ctx.enter_context(tc.tile_pool(name="vb", bufs=1))
        epool = ctx.enter_context(tc.tile_pool(name="eb", bufs=1))
        opool = ctx.enter_context(tc.tile_pool(name="ob", bufs=1))
        rzpool = ctx.enter_context(tc.tile_pool(name="rz", bufs=2))
        outpool = ctx.enter_context(tc.tile_pool(name="outb", bufs=4))
        spool = ctx.enter_context(tc.tile_pool(name="small", bufs=2))
        pmm = ctx.enter_context(tc.tile_pool(name="pmm", bufs=4, space="PSUM"))
        pzb = ctx.enter_context(tc.tile_pool(name="pzb", bufs=2, space="PSUM"))

        # ---- constants ----
        ones_mat = cpool.tile([P, P], F32R)
        nc.sync.dma_start(ones_mat[:], aps["ones"][:])
        amat_sb = cpool.tile([P, P], F32)
        nc.sync.dma_start(amat_sb[:], amat[:])
        prm_sb = cpool.tile([P, 6, CT], F32)
        nc.sync.dma_start(prm_sb[:], prm.rearrange("q (t p) -> p q t", p=P))

        # ---- weights: [c_in, c_out] -> [p, kt, c_out] ----
        w_sb = {}
        for name, ap in (("wq", wq), ("wk", wk), ("wv", wv), ("wp", wp)):
            t = wpool.tile([P, CT, C], F32R, tag=name)
            nc.sync.dma_start(t[:], ap.rearrange("(t p) o -> p t o", p=P))
            w_sb[name] = t

        def groupnorm(src_sb, dst_sb, scale_row, bias_row, uid, dst_is_f32=False):
            """dst = GroupNorm(src) with affine params from prm rows.

            The normalized output always feeds fp32r matmuls, so the write
            is fp32r-typed (the engine rounds on write); dst_is_f32 marks an
            fp32-allocated destination that needs the bitcast view."""
            stats = spool.tile([P, CT, 3], F32, tag=f"st{uid}")
            for t in range(CT):
                bns = spool.tile([P, 2, 6], F32, tag=f"bns{uid}")
                for h2 in range(2):
                    nc.vector.bn_stats(
                        bns[:, h2, :], src_sb[:, t, h2 * 512 : (h2 + 1) * 512]
                    )
                nc.vector.bn_aggr(stats[:, t, 0:2], bns[:])
                # mean^2 for the cross-partition variance law
                nc.vector.tensor_tensor(
                    stats[:, t, 2:3], stats[:, t, 0:1], stats[:, t, 0:1],
                    op=ALU.mult,
                )
            # group-average (and broadcast) of [mean, var, mean^2] over each
            # 16-partition group: amat is block-diagonal ones/16.
            gps = pmm.tile([P, CT, 3], F32, tag="ps")
            nc.tensor.matmul(gps[:], amat_sb[:], stats[:], start=True, stop=True)
            g = spool.tile([P, CT, 3], F32, tag=f"g{uid}")
            nc.scalar.copy(g[:], gps[:])
            # var_g = E[var] + E[mean^2] - E[mean]^2  (equal-count partitions)
            msq = spool.tile([P, CT], F32, tag=f"msq{uid}")
            nc.vector.tensor_tensor(msq[:], g[:, :, 0], g[:, :, 0], op=ALU.mult)
            var = spool.tile([P, CT], F32, tag=f"var{uid}")
            nc.vector.tensor_tensor(var[:], g[:, :, 1], g[:, :, 2], op=ALU.add)
            nc.vector.tensor_tensor(var[:], var[:], msq[:], op=ALU.subtract)
            nc.vector.tensor_scalar(var[:], var[:], EPS, None, op0=ALU.add)
            # rstd = 1/sqrt(var+eps), Newton-polished
            std = spool.tile([P, CT], F32, tag=f"std{uid}")
            nc.scalar.activation(std[:], var[:], AF.Sqrt)
            r0 = spool.tile([P, CT], F32, tag=f"r0{uid}")
            nc.vector.reciprocal(r0[:], std[:])
            t7 = spool.tile([P, CT], F32, tag=f"t7{uid}")
            nc.vector.tensor_tensor(t7[:], r0[:], r0[:], op=ALU.mult)
            nc.vector.tensor_tensor(t7[:], var[:], t7[:], op=ALU.mult)
            nc.vector.tensor_scalar(t7[:], t7[:], -0.5, 1.5, op0=ALU.mult, op1=ALU.add)
            rstd = spool.tile([P, CT], F32, tag=f"rs{uid}")
            nc.vector.tensor_tensor(rstd[:], r0[:], t7[:], op=ALU.mult)
            # a = rstd*gamma ; b = beta - mean*a ; dst = src*a + b
            a = spool.tile([P, CT], F32, tag=f"a{uid}")
            nc.vector.tensor_tensor(a[:], rstd[:], prm_sb[:, scale_row, :], op=ALU.mult)
            mb = spool.tile([P, CT], F32, tag=f"mb{uid}")
            nc.vector.tensor_tensor(mb[:], g[:, :, 0], a[:], op=ALU.mult)
            nc.vector.tensor_tensor(mb[:], prm_sb[:, bias_row, :], mb[:], op=ALU.subtract)
            for t in range(CT):
                dst = dst_sb[:, t, :]
                if dst_is_f32:
                    dst = _r(dst)
                nc.vector.tensor_scalar(
                    dst, src_sb[:, t, :],
                    a[:, t : t + 1], mb[:, t : t + 1],
                    op0=ALU.mult, op1=ALU.add,
                )

        for b in range(BPC):
            outv = out[b].rearrange("(t p) n -> p t n", p=P)

            x_sb = xpool.tile([P, CT, HW], F32, tag="x")
            nc.sync.dma_start(x_sb[:], xs[b].rearrange("(t p) n -> p t n", p=P))
            y_sb = ypool.tile([P, CT, HW], F32, tag="y")
            nc.sync.dma_start(y_sb[:], ys[b].rearrange("(t p) n -> p t n", p=P))

            h_sb = hpool.tile([P, CT, HW], F32R, tag="h")
            groupnorm(x_sb, h_sb, R_GN_SCALE, R_GN_BIAS, uid=f"x{b}")
            yn_sb = ynpool.tile([P, CT, HW], F32R, tag="yn")
            # y_ normalized in place
            groupnorm(y_sb, yn_sb, R_GN1_SCALE, R_GN1_BIAS, uid=f"y{b}")

            # ---- k = Wk h  (k[c_out, i]) ----
            k_sb = kpool.tile([P, CT, HW], F32R, tag="k")
            for mt in range(CT):
                for nh in range(IB):
                    ps = pmm.tile([P, IBS], F32, tag="ps")
                    for kt in range(CT):
                        nc.tensor.matmul(
                            ps[:],
                            w_sb["wk"][:, kt, mt * P : (mt + 1) * P],
                            h_sb[:, kt, nh * IBS : (nh + 1) * IBS],
                            start=(kt == 0), stop=(kt == CT - 1),
                        )
                    nc.scalar.copy(k_sb[:, mt, nh * IBS : (nh + 1) * IBS], ps[:])

            # ---- vT[j, c_out] = h^T WvT ----
            vT_sb = vpool.tile([P, JT, C], F32R, tag="vT")
            for jt in range(JT):
                ps = pmm.tile([P, C], F32, tag="ps")
                for kt in range(CT):
                    nc.tensor.matmul(
                        ps[:],
                        h_sb[:, kt, jt * P : (jt + 1) * P],
                        w_sb["wv"][:, kt, :],
                        start=(kt == 0), stop=(kt == CT - 1),
                    )
                nc.scalar.copy(vT_sb[:, jt, :], ps[:])

            # ---- q = Wq y_ + bq ----
            q_sb = qpool.tile([P, CT, HW], F32R, tag="q")
            for mt in range(CT):
                for nh in range(IB):
                    ps = pmm.tile([P, IBS], F32, tag="ps")
                    for kt in range(CT):
                        nc.tensor.matmul(
                            ps[:],
                            w_sb["wq"][:, kt, mt * P : (mt + 1) * P],
                            yn_sb[:, kt, nh * IBS : (nh + 1) * IBS],
                            start=(kt == 0), stop=(kt == CT - 1),
                        )
                    nc.vector.tensor_scalar(
                        q_sb[:, mt, nh * IBS : (nh + 1) * IBS], ps[:],
                        prm_sb[:, R_BQ, mt : mt + 1], None, op0=ALU.add,
                    )

            # ---- attention, one i-block (512 queries) at a time ----
            for ib in range(IB):
                isl = slice(ib * IBS, (ib + 1) * IBS)
                e_sb = epool.tile([P, JT, IBS], F32R, tag="e")
                zb = pzb.tile([P, IBS], F32, tag="zb")
                for jt in range(JT):
                    ps = pmm.tile([P, IBS], F32, tag="ps")
                    for kt in range(CT):
                        nc.tensor.matmul(
                            ps[:],
                            k_sb[:, kt, jt * P : (jt + 1) * P],
                            q_sb[:, kt, isl],
                            start=(kt == 0), stop=(kt == CT - 1),
                        )
                    # E = exp(S / sqrt(c)); logits are O(1), no max needed
                    nc.scalar.activation(e_sb[:, jt, :], ps[:], AF.Exp, scale=SM_SCALE)
                    # Z[p, i] = sum_j E[j, i] for every p (ones matmul broadcast)
                    nc.tensor.matmul(
                        zb[:], ones_mat[:], e_sb[:, jt, :],
                        start=(jt == 0), stop=(jt == JT - 1),
                    )
                rzb = rzpool.tile([P, IBS], F32, tag="rzb")
                nc.vector.reciprocal(rzb[:], zb[:])

                o0_sb = opool.tile([P, CT, IBS], F32R, tag="o0")
                for ct in range(CT):
                    ps = pmm.tile([P, IBS], F32, tag="ps")
                    for jt in range(JT):
                        nc.tensor.matmul(
                            ps[:],
                            vT_sb[:, jt, ct * P : (ct + 1) * P],
                            e_sb[:, jt, :],
                            start=(jt == 0), stop=(jt == JT - 1),
                        )
                    nc.vector.tensor_tensor(o0_sb[:, ct, :], ps[:], rzb[:], op=ALU.mult)

                # ---- r = Wp o0n + bp' ; out = x + r ----
                for mt in range(CT):
                    ps = pmm.tile([P, IBS], F32, tag="ps")
                    for ct in range(CT):
                        nc.tensor.matmul(
                            ps[:],
                            w_sb["wp"][:, ct, mt * P : (mt + 1) * P],
                            o0_sb[:, ct, :],
                            start=(ct == 0), stop=(ct == CT - 1),
                        )
                    ot = outpool.tile([P, IBS], F32, tag="ot")
                    nc.vector.scalar_tensor_tensor(
                        ot[:], ps[:], prm_sb[:, R_BPP, mt : mt + 1],
                        x_sb[:, mt, isl], op0=ALU.add, op1=ALU.add,
                    )
                    nc.sync.dma_start(outv[:, mt, isl], ot[:])


_CACHE = {}


def _build():
    if "nc" in _CACHE:
        return _CACHE["nc"]
    nc = bacc.Bacc("TRN2", target_bir_lowering=False, debug=False)
    aps = {
        "xs": nc.dram_tensor("xs", [BPC, C, HW], F32, kind="ExternalInput").ap(),
        "ys": nc.dram_tensor("ys", [BPC, C, HW], F32, kind="ExternalInput").ap(),
        "wqT": nc.dram_tensor("wqT", [C, C], F32R, kind="ExternalInput").ap(),
        "wkT": nc.dram_tensor("wkT", [C, C], F32R, kind="ExternalInput").ap(),
        "wvT": nc.dram_tensor("wvT", [C, C], F32R, kind="ExternalInput").ap(),
        "wpT": nc.dram_tensor("wpT", [C, C], F32R, kind="ExternalInput").ap(),
        "prm": nc.dram_tensor("prm", [6, C], F32, kind="ExternalInput").ap(),
        "amat": nc.dram_tensor("amat", [P, P], F32, kind="ExternalInput").ap(),
        "ones": nc.dram_tensor("ones", [P, P], F32R, kind="ExternalInput").ap(),
        "out": nc.dram_tensor("out", [BPC, C, HW], F32, kind="ExternalOutput").ap(),
    }
    with tile.TileContext(nc) as tc:
        _emit(tc, aps)
    nc.compile()
    _CACHE["nc"] = nc
    return nc


def _round_fp32r(a):
    """Round fp32 to the PE's fp32r format: 1+8+11 bits, low 12 zeroed."""
    u = np.ascontiguousarray(a, dtype=np.float32).view(np.uint32)
    u = u + np.uint32(0x7FF) + ((u >> np.uint32(12)) & np.uint32(1))
    u = u & np.uint32(0xFFFFF000)
    return u.view(np.float32)


def _host_inputs(x, y, norm_scale, norm_bias, norm1_scale, norm1_bias,
                 wq, bq, wk, bk, wv, bv, wp, bp):
    f = lambda a: np.ascontiguousarray(np.asarray(a, dtype=np.float32))
    x = f(x).reshape(B, C, HW)
    y = f(y).reshape(B, C, HW)
    wq, wk, wv, wp = f(wq), f(wk), f(wv), f(wp)
    # bk cancels in softmax; bv folds into bp' because softmax rows sum to 1
    bpp = f(bp) + wp @ f(bv)
    prm = np.ascontiguousarray(
        np.stack([f(norm_scale), f(norm_bias), f(norm1_scale), f(norm1_bias),
                  f(bq), bpp]).astype(np.float32)
    )
    amat = np.zeros((P, P), np.float32)
    for g in range(P // GSIZE):
        amat[g * GSIZE : (g + 1) * GSIZE, g * GSIZE : (g + 1) * GSIZE] = 1.0 / GSIZE
    shared = {
        "wqT": _round_fp32r(wq.T), "wkT": _round_fp32r(wk.T),
        "wvT": _round_fp32r(wv.T), "wpT": _round_fp32r(wp.T),
        "prm": prm, "amat": amat, "ones": np.ones((P, P), np.float32),
    }
    in_maps = []
    for core in range(NCORES):
        sl = slice(core * BPC, (core + 1) * BPC)
        in_maps.append({
            "xs": np.ascontiguousarray(x[sl]),
            "ys": np.ascontiguousarray(y[sl]),
            **shared,
        })
    return in_maps


def _run(in_maps, trace=False):
    nc = _build()
    res = run_bass_kernel_spmd(
        nc, in_maps, core_ids=list(range(NCORES)), trace=trace
    )
    out = np.concatenate(
        [res.results[i]["out"] for i in range(NCORES)], axis=0
    ).reshape(B, C, H, W)
    return out, res


def kernel(**inputs):
    in_maps = _host_inputs(**inputs)
    out, _ = _run(in_maps, trace=False)
    return out


# revision 10
# speedup vs baseline: 1.0551x; 1.0551x over previous
"""AttnBlock fusion kernel for Trainium2 (Bass/Tile), 8 NeuronCores.

Reference computation (per batch element b; c=512 channels, hw=1024 spatial):
    h  = GroupNorm(32, c)(x) ; k = Wk h + bk ; v = Wv h + bv
    y_ = GroupNorm(32, c)(y) ; q = Wq y_ + bq
    attn = softmax_j(q^T k / sqrt(c)) ; o = v @ attn^T ; out = x + Wp o + bp

Sharding: pure data parallel over batch (16 batches / 8 cores = 2 each).

Key algebraic moves (exactness notes inline):
  * bk cancels in softmax over j (adds a per-i constant to logits) -> dropped.
  * bv contributes bv * sum_j(attn) = bv exactly -> folded into
    bp' = bp + Wp @ bv on the host.
  * v is produced directly transposed (vT[j,c]) by the projection matmul,
    and attention scores are computed as S[j,i]; no on-chip transposes.
  * softmax denominator: ones[128,128] matmul gives the partition-broadcast
    column sum of exp(S) directly in PSUM.
All matmuls run in float32r (full fp32 storage, 1 cycle/row at N=512).
"""

import math
import os
import sys

import numpy as np

for _p in ("/opt/trn_rl_repo", "/root/.axon_site/_ro/trn_rl_repo"):
    if os.path.isdir(_p) and _p not in sys.path:
        sys.path.append(_p)

import concourse.bass as bass
import concourse.bacc as bacc
import concourse.mybir as mybir
import concourse.tile as tile
from concourse.bass_utils import run_bass_kernel_spmd

F32 = mybir.dt.float32
F32R = mybir.dt.float32r
AF = mybir.ActivationFunctionType
ALU = mybir.AluOpType

B, C, H, W = 16, 512, 32, 32
HW = H * W                  # 1024
NCORES = 8
BPC = B // NCORES           # 2 batches per core
P = 128                     # SBUF partitions
CT = C // P                 # 4 channel tiles
JT = HW // P                # 8 key-position tiles
IBS = 512                   # i-block size (query positions per block)
IB = HW // IBS              # 2 i-blocks
GROUPS = 32
GSIZE = C // GROUPS         # 16 channels per group
EPS = 1e-6
SM_SCALE = float(int(C) ** -0.5)

# prm rows
R_GN_SCALE, R_GN_BIAS, R_GN1_SCALE, R_GN1_BIAS, R_BQ, R_BPP = range(6)


def _r(ap):
    """View an fp32 AP as float32r for the tensor engine."""
    return ap.bitcast(F32R)


def _emit(tc, aps):
    nc = tc.nc
    xs, ys, wq, wk, wv, wp, prm, amat, out = (
        aps["xs"], aps["ys"], aps["wqT"], aps["wkT"], aps["wvT"], aps["wpT"],
        aps["prm"], aps["amat"], aps["out"],
    )

    from contextlib import ExitStack

    with ExitStack() as ctx:
        cpool = ctx.enter_context(tc.tile_pool(name="const", bufs=1))
        wpool = ctx.enter_context(tc.tile_pool(name="w", bufs=1))
        xpool = ctx.enter_context(tc.tile_pool(name="xin", bufs=2))
        ypool = ctx.enter_context(tc.tile_pool(name="yin", bufs=1))
        ynpool = ctx.enter_context(tc.tile_pool(name="yn", bufs=1))
        hpool = ctx.enter_context(tc.tile_pool(name="hb", bufs=1))
        qpool = ctx.enter_context(tc.tile_pool(name="qb", bufs=1))
        kpool = ctx.enter_context(tc.tile_pool(name="kb", bufs=1))
        vpool = ctx.enter_context(tc.tile_pool(name="vb", bufs=1))
        epool = ctx.enter_context(tc.tile_pool(name="eb", bufs=1))
        opool = ctx.enter_context(tc.tile_pool(name="ob", bufs=1))
        rzpool = ctx.enter_context(tc.tile_pool(name="rz", bufs=2))
        outpool = ctx.enter_context(tc.tile_pool(name="outb", bufs=4))
        spool = ctx.enter_context(tc.tile_pool(name="small", bufs=2))
        pmm = ctx.enter_context(tc.tile_pool(name="pmm", bufs=4, space="PSUM"))
        pzb = ctx.enter_context(tc.tile_pool(name="pzb", bufs=2, space="PSUM"))

        # ---- constants ----
        ones_mat = cpool.tile([P, P], F32R)
        nc.sync.dma_start(ones_mat[:], aps["ones"][:])
        amat_sb = cpool.tile([P, P], F32)
        nc.sync.dma_start(amat_sb[:], amat[:])
        prm_sb = cpool.tile([P, 6, CT], F32)
        nc.sync.dma_start(prm_sb[:], prm.rearrange("p (q t) -> p q t", t=CT))

        # ---- weights: [c_in, c_out] -> [p, kt, c_out] ----
        w_sb = {}
        for name, ap in (("wq", wq), ("wk", wk), ("wv", wv), ("wp", wp)):
            t = wpool.tile([P, CT, C], F32R, tag=name)
            nc.sync.dma_start(t[:], ap.rearrange("p (t o) -> p t o", o=C))
            w_sb[name] = t

        def groupnorm(src_sb, dst_sb, scale_row, bias_row, uid, dst_is_f32=False):
            """dst = GroupNorm(src) with affine params from prm rows.

            The normalized output always feeds fp32r matmuls, so the write
            is fp32r-typed (the engine rounds on write); dst_is_f32 marks an
            fp32-allocated destination that needs the bitcast view."""
            stats = spool.tile([P, CT, 3], F32, tag=f"st{uid}")
            for t in range(CT):
                bns = spool.tile([P, 2, 6], F32, tag=f"bns{uid}")
                for h2 in range(2):
                    nc.vector.bn_stats(
                        bns[:, h2, :], src_sb[:, t, h2 * 512 : (h2 + 1) * 512]
                    )
                nc.vector.bn_aggr(stats[:, t, 0:2], bns[:])
                # mean^2 for the cross-partition variance law
                nc.vector.tensor_tensor(
                    stats[:, t, 2:3], stats[:, t, 0:1], stats[:, t, 0:1],
                    op=ALU.mult,
                )
            # group-average (and broadcast) of [mean, var, mean^2] over each
            # 16-partition group: amat is block-diagonal ones/16.
            gps = pmm.tile([P, CT, 3], F32, tag="ps")
            nc.tensor.matmul(gps[:], amat_sb[:], stats[:], start=True, stop=True)
            g = spool.tile([P, CT, 3], F32, tag=f"g{uid}")
            nc.scalar.copy(g[:], gps[:])
            # var_g = E[var] + E[mean^2] - E[mean]^2  (equal-count partitions)
            msq = spool.tile([P, CT], F32, tag=f"msq{uid}")
            nc.vector.tensor_tensor(msq[:], g[:, :, 0], g[:, :, 0], op=ALU.mult)
            var = spool.tile([P, CT], F32, tag=f"var{uid}")
            nc.vector.tensor_tensor(var[:], g[:, :, 1], g[:, :, 2], op=ALU.add)
            nc.vector.tensor_tensor(var[:], var[:], msq[:], op=ALU.subtract)
            nc.vector.tensor_scalar(var[:], var[:], EPS, None, op0=ALU.add)
            # rstd = 1/sqrt(var+eps), Newton-polished
            std = spool.tile([P, CT], F32, tag=f"std{uid}")
            nc.scalar.activation(std[:], var[:], AF.Sqrt)
            r0 = spool.tile([P, CT], F32, tag=f"r0{uid}")
            nc.vector.reciprocal(r0[:], std[:])
            t7 = spool.tile([P, CT], F32, tag=f"t7{uid}")
            nc.vector.tensor_tensor(t7[:], r0[:], r0[:], op=ALU.mult)
            nc.vector.tensor_tensor(t7[:], var[:], t7[:], op=ALU.mult)
            nc.vector.tensor_scalar(t7[:], t7[:], -0.5, 1.5, op0=ALU.mult, op1=ALU.add)
            rstd = spool.tile([P, CT], F32, tag=f"rs{uid}")
            nc.vector.tensor_tensor(rstd[:], r0[:], t7[:], op=ALU.mult)
            # a = rstd*gamma ; b = beta - mean*a ; dst = src*a + b
            a = spool.tile([P, CT], F32, tag=f"a{uid}")
            nc.vector.tensor_tensor(a[:], rstd[:], prm_sb[:, scale_row, :], op=ALU.mult)
            mb = spool.tile([P, CT], F32, tag=f"mb{uid}")
            nc.vector.tensor_tensor(mb[:], g[:, :, 0], a[:], op=ALU.mult)
            nc.vector.tensor_tensor(mb[:], prm_sb[:, bias_row, :], mb[:], op=ALU.subtract)
            for t in range(CT):
                dst = dst_sb[:, t, :]
                if dst_is_f32:
                    dst = _r(dst)
                nc.vector.tensor_scalar(
                    dst, src_sb[:, t, :],
                    a[:, t : t + 1], mb[:, t : t + 1],
                    op0=ALU.mult, op1=ALU.add,
                )

        for b in range(BPC):
            outv = out[b].rearrange("p (t n) -> p t n", n=HW)

            x_sb = xpool.tile([P, CT, HW], F32, tag="x")
            nc.sync.dma_start(x_sb[:], xs[b].rearrange("p (t n) -> p t n", n=HW))
            y_sb = ypool.tile([P, CT, HW], F32, tag="y")
            nc.sync.dma_start(y_sb[:], ys[b].rearrange("p (t n) -> p t n", n=HW))

            h_sb = hpool.tile([P, CT, HW], F32R, tag="h")
            groupnorm(x_sb, h_sb, R_GN_SCALE, R_GN_BIAS, uid=f"x{b}")
            yn_sb = ynpool.tile([P, CT, HW], F32R, tag="yn")
            # y_ normalized in place
            groupnorm(y_sb, yn_sb, R_GN1_SCALE, R_GN1_BIAS, uid=f"y{b}")

            # ---- k = Wk h  (k[c_out, i]) ----
            k_sb = kpool.tile([P, CT, HW], F32R, tag="k")
            for mt in range(CT):
                for nh in range(IB):
                    ps = pmm.tile([P, IBS], F32, tag="ps")
                    for kt in range(CT):
                        nc.tensor.matmul(
                            ps[:],
                            w_sb["wk"][:, kt, mt * P : (mt + 1) * P],
                            h_sb[:, kt, nh * IBS : (nh + 1) * IBS],
                            start=(kt == 0), stop=(kt == CT - 1),
                        )
                    nc.scalar.copy(k_sb[:, mt, nh * IBS : (nh + 1) * IBS], ps[:])

            # ---- vT[j, c_out] = h^T WvT ----
            vT_sb = vpool.tile([P, JT, C], F32R, tag="vT")
            for jt in range(JT):
                ps = pmm.tile([P, C], F32, tag="ps")
                for kt in range(CT):
                    nc.tensor.matmul(
                        ps[:],
                        h_sb[:, kt, jt * P : (jt + 1) * P],
                        w_sb["wv"][:, kt, :],
                        start=(kt == 0), stop=(kt == CT - 1),
                    )
                nc.scalar.copy(vT_sb[:, jt, :], ps[:])

            # ---- q = Wq y_ + bq ----
            q_sb = qpool.tile([P, CT, HW], F32R, tag="q")
            for mt in range(CT):
                for nh in range(IB):
                    ps = pmm.tile([P, IBS], F32, tag="ps")
                    for kt in range(CT):
                        nc.tensor.matmul(
                            ps[:],
                            w_sb["wq"][:, kt, mt * P : (mt + 1) * P],
                            yn_sb[:, kt, nh * IBS : (nh + 1) * IBS],
                            start=(kt == 0), stop=(kt == CT - 1),
                        )
                    nc.vector.tensor_scalar(
                        q_sb[:, mt, nh * IBS : (nh + 1) * IBS], ps[:],
                        prm_sb[:, R_BQ, mt : mt + 1], None, op0=ALU.add,
                    )

            # ---- attention, one i-block (512 queries) at a time ----
            for ib in range(IB):
                isl = slice(ib * IBS, (ib + 1) * IBS)
                e_sb = epool.tile([P, JT, IBS], F32R, tag="e")
                zb = pzb.tile([P, IBS], F32, tag="zb")
                for jt in range(JT):
                    ps = pmm.tile([P, IBS], F32, tag="ps")
                    for kt in range(CT):
                        nc.tensor.matmul(
                            ps[:],
                            k_sb[:, kt, jt * P : (jt + 1) * P],
                            q_sb[:, kt, isl],
                            start=(kt == 0), stop=(kt == CT - 1),
                        )
                    # E = exp(S / sqrt(c)); logits are O(1), no max needed
                    nc.scalar.activation(e_sb[:, jt, :], ps[:], AF.Exp, scale=SM_SCALE)
                    # Z[p, i] = sum_j E[j, i] for every p (ones matmul broadcast)
                    nc.tensor.matmul(
                        zb[:], ones_mat[:], e_sb[:, jt, :],
                        start=(jt == 0), stop=(jt == JT - 1),
                    )
                rzb = rzpool.tile([P, IBS], F32, tag="rzb")
                nc.vector.reciprocal(rzb[:], zb[:])

                o0_sb = opool.tile([P, CT, IBS], F32R, tag="o0")
                for ct in range(CT):
                    ps = pmm.tile([P, IBS], F32, tag="ps")
                    for jt in range(JT):
                        nc.tensor.matmul(
                            ps[:],
                            vT_sb[:, jt, ct * P : (ct + 1) * P],
                            e_sb[:, jt, :],
                            start=(jt == 0), stop=(jt == JT - 1),
                        )
                    nc.vector.tensor_tensor(o0_sb[:, ct, :], ps[:], rzb[:], op=ALU.mult)

                # ---- r = Wp o0n + bp' ; out = x + r ----
                for mt in range(CT):
                    ps = pmm.tile([P, IBS], F32, tag="ps")
                    for ct in range(CT):
                        nc.tensor.matmul(
                            ps[:],
                            w_sb["wp"][:, ct, mt * P : (mt + 1) * P],
                            o0_sb[:, ct, :],
                            start=(ct == 0), stop=(ct == CT - 1),
                        )
                    ot = outpool.tile([P, IBS], F32, tag="ot")
                    nc.vector.scalar_tensor_tensor(
                        ot[:], ps[:], prm_sb[:, R_BPP, mt : mt + 1],
                        x_sb[:, mt, isl], op0=ALU.add, op1=ALU.add,
                    )
                    nc.sync.dma_start(outv[:, mt, isl], ot[:])


_CACHE = {}


def _build():
    if "nc" in _CACHE:
        return _CACHE["nc"]
    nc = bacc.Bacc("TRN2", target_bir_lowering=False, debug=False)
    aps = {
        "xs": nc.dram_tensor("xs", [BPC, P, CT * HW], F32, kind="ExternalInput").ap(),
        "ys": nc.dram_tensor("ys", [BPC, P, CT * HW], F32, kind="ExternalInput").ap(),
        "wqT": nc.dram_tensor("wqT", [P, CT * C], F32R, kind="ExternalInput").ap(),
        "wkT": nc.dram_tensor("wkT", [P, CT * C], F32R, kind="ExternalInput").ap(),
        "wvT": nc.dram_tensor("wvT", [P, CT * C], F32R, kind="ExternalInput").ap(),
        "wpT": nc.dram_tensor("wpT", [P, CT * C], F32R, kind="ExternalInput").ap(),
        "prm": nc.dram_tensor("prm", [P, 6 * CT], F32, kind="ExternalInput").ap(),
        "amat": nc.dram_tensor("amat", [P, P], F32, kind="ExternalInput").ap(),
        "ones": nc.dram_tensor("ones", [P, P], F32R, kind="ExternalInput").ap(),
        "out": nc.dram_tensor("out", [BPC, P, CT * HW], F32, kind="ExternalOutput").ap(),
    }
    with tile.TileContext(nc) as tc:
        _emit(tc, aps)
    nc.compile()
    _CACHE["nc"] = nc
    return nc


def _pack_chw(a):
    """[*, C, HW] -> [*, P, CT*HW] matching SBUF layout c = t*128 + p."""
    lead = a.shape[:-2]
    a = a.reshape(*lead, CT, P, HW)
    a = np.moveaxis(a, -3, -2)          # [..., P, CT, HW]
    return np.ascontiguousarray(a.reshape(*lead, P, CT * HW))


def _unpack_chw(a):
    """[*, P, CT*HW] -> [*, C, HW]."""
    lead = a.shape[:-2]
    a = a.reshape(*lead, P, CT, HW)
    a = np.moveaxis(a, -2, -3)          # [..., CT, P, HW]
    return np.ascontiguousarray(a.reshape(*lead, CT * P, HW))


def _round_fp32r(a):
    """Round fp32 to the PE's fp32r format: 1+8+11 bits, low 12 zeroed."""
    u = np.ascontiguousarray(a, dtype=np.float32).view(np.uint32)
    u = u + np.uint32(0x7FF) + ((u >> np.uint32(12)) & np.uint32(1))
    u = u & np.uint32(0xFFFFF000)
    return u.view(np.float32)


def _host_inputs(x, y, norm_scale, norm_bias, norm1_scale, norm1_bias,
                 wq, bq, wk, bk, wv, bv, wp, bp):
    f = lambda a: np.ascontiguousarray(np.asarray(a, dtype=np.float32))
    x = f(x).reshape(B, C, HW)
    y = f(y).reshape(B, C, HW)
    wq, wk, wv, wp = f(wq), f(wk), f(wv), f(wp)
    # bk cancels in softmax; bv folds into bp' because softmax rows sum to 1
    bpp = f(bp) + wp @ f(bv)
    prm = np.stack([f(norm_scale), f(norm_bias), f(norm1_scale), f(norm1_bias),
                    f(bq), bpp]).astype(np.float32)
    # [6, C] -> [P, 6*CT] matching prm_sb[p, q, t]
    prm = np.ascontiguousarray(
        prm.reshape(6, CT, P).transpose(2, 0, 1).reshape(P, 6 * CT)
    )
    amat = np.zeros((P, P), np.float32)
    for g in range(P // GSIZE):
        amat[g * GSIZE : (g + 1) * GSIZE, g * GSIZE : (g + 1) * GSIZE] = 1.0 / GSIZE
    def packw(w):
        # wT [c_in, c_out] -> [P, CT*C] matching w_sb[p, kt, o]
        wT = _round_fp32r(w.T)
        return np.ascontiguousarray(
            wT.reshape(CT, P, C).transpose(1, 0, 2).reshape(P, CT * C)
        )

    shared = {
        "wqT": packw(wq), "wkT": packw(wk), "wvT": packw(wv), "wpT": packw(wp),
        "prm": prm, "amat": amat, "ones": np.ones((P, P), np.float32),
    }
    in_maps = []
    for core in range(NCORES):
        sl = slice(core * BPC, (core + 1) * BPC)
        in_maps.append({
            "xs": _pack_chw(x[sl]),
            "ys": _pack_chw(y[sl]),
            **shared,
        })
    return in_maps


def _run(in_maps, trace=False):
    nc = _build()
    res = run_bass_kernel_spmd(
        nc, in_maps, core_ids=list(range(NCORES)), trace=trace
    )
    out = np.concatenate(
        [_unpack_chw(res.results[i]["out"]) for i in range(NCORES)], axis=0
    ).reshape(B, C, H, W)
    return out, res


def kernel(**inputs):
    in_maps = _host_inputs(**inputs)
    out, _ = _run(in_maps, trace=False)
    return out


# revision 11
# speedup vs baseline: 1.1077x; 1.0499x over previous
"""AttnBlock fusion kernel for Trainium2 (Bass/Tile), 8 NeuronCores.

Reference computation (per batch element b; c=512 channels, hw=1024 spatial):
    h  = GroupNorm(32, c)(x) ; k = Wk h + bk ; v = Wv h + bv
    y_ = GroupNorm(32, c)(y) ; q = Wq y_ + bq
    attn = softmax_j(q^T k / sqrt(c)) ; o = v @ attn^T ; out = x + Wp o + bp

Sharding: pure data parallel over batch (16 batches / 8 cores = 2 each).

Key algebraic moves (exactness notes inline):
  * bk cancels in softmax over j (adds a per-i constant to logits) -> dropped.
  * bv contributes bv * sum_j(attn) = bv exactly -> folded into
    bp' = bp + Wp @ bv on the host.
  * v is produced directly transposed (vT[j,c]) by the projection matmul,
    and attention scores are computed as S[j,i]; no on-chip transposes.
  * softmax denominator: ones[128,128] matmul gives the partition-broadcast
    column sum of exp(S) directly in PSUM.
All matmuls run in float32r (full fp32 storage, 1 cycle/row at N=512).
"""

import math
import os
import sys

import numpy as np

for _p in ("/opt/trn_rl_repo", "/root/.axon_site/_ro/trn_rl_repo"):
    if os.path.isdir(_p) and _p not in sys.path:
        sys.path.append(_p)

import concourse.bass as bass
import concourse.bacc as bacc
import concourse.mybir as mybir
import concourse.tile as tile
from concourse.bass_utils import run_bass_kernel_spmd

F32 = mybir.dt.float32
F32R = mybir.dt.float32r
AF = mybir.ActivationFunctionType
ALU = mybir.AluOpType

B, C, H, W = 16, 512, 32, 32
HW = H * W                  # 1024
NCORES = 8
BPC = B // NCORES           # 2 batches per core
P = 128                     # SBUF partitions
CT = C // P                 # 4 channel tiles
JT = HW // P                # 8 key-position tiles
IBS = 512                   # i-block size (query positions per block)
IB = HW // IBS              # 2 i-blocks
GROUPS = 32
GSIZE = C // GROUPS         # 16 channels per group
EPS = 1e-6
SM_SCALE = float(int(C) ** -0.5)

# prm rows
R_GN_SCALE, R_GN_BIAS, R_GN1_SCALE, R_GN1_BIAS, R_BQ, R_BPP = range(6)


def _r(ap):
    """View an fp32 AP as float32r for the tensor engine."""
    return ap.bitcast(F32R)


def _emit(tc, aps):
    nc = tc.nc
    xs, ys, wq, wk, wv, wp, prm, amat, out = (
        aps["xs"], aps["ys"], aps["wqT"], aps["wkT"], aps["wvT"], aps["wpT"],
        aps["prm"], aps["amat"], aps["out"],
    )

    from contextlib import ExitStack

    with ExitStack() as ctx:
        cpool = ctx.enter_context(tc.tile_pool(name="const", bufs=1))
        wpool = ctx.enter_context(tc.tile_pool(name="w", bufs=1))
        xpool = ctx.enter_context(tc.tile_pool(name="xin", bufs=2))
        ypool = ctx.enter_context(tc.tile_pool(name="yin", bufs=1))
        ynpool = ctx.enter_context(tc.tile_pool(name="yn", bufs=1))
        hpool = ctx.enter_context(tc.tile_pool(name="hb", bufs=1))
        qpool = ctx.enter_context(tc.tile_pool(name="qb", bufs=1))
        kpool = ctx.enter_context(tc.tile_pool(name="kb", bufs=1))
        vpool = ctx.enter_context(tc.tile_pool(name="vb", bufs=1))
        epool = ctx.enter_context(tc.tile_pool(name="eb", bufs=1))
        opool = ctx.enter_context(tc.tile_pool(name="ob", bufs=1))
        rzpool = ctx.enter_context(tc.tile_pool(name="rz", bufs=2))
        outpool = ctx.enter_context(tc.tile_pool(name="outb", bufs=4))
        spool = ctx.enter_context(tc.tile_pool(name="small", bufs=2))
        pmm = ctx.enter_context(tc.tile_pool(name="pmm", bufs=5, space="PSUM"))
        pzb = ctx.enter_context(tc.tile_pool(name="pzb", bufs=2, space="PSUM"))

        def load_xy(b):
            """Per-tile DMAs so GN stats can start as each tile lands."""
            x_sb = xpool.tile([P, CT, HW], F32, tag="x")
            y_sb = ypool.tile([P, CT, HW], F32, tag="y")
            for t in range(CT):
                nc.sync.dma_start(
                    x_sb[:, t, :], xs[b].rearrange("p (t n) -> p t n", n=HW)[:, t, :]
                )
            for t in range(CT):
                nc.sync.dma_start(
                    y_sb[:, t, :], ys[b].rearrange("p (t n) -> p t n", n=HW)[:, t, :]
                )
            return x_sb, y_sb

        def gn_stats(src_sb, scale_row, bias_row, uid):
            """Per-(batch,input) GroupNorm scale/shift: a, mb tiles [P, CT]."""
            stats = spool.tile([P, CT, 3], F32, tag=f"st{uid}")
            for t in range(CT):
                bns = spool.tile([P, 2, 6], F32, tag=f"bns{uid}")
                for h2 in range(2):
                    nc.vector.bn_stats(
                        bns[:, h2, :], src_sb[:, t, h2 * 512 : (h2 + 1) * 512]
                    )
                nc.vector.bn_aggr(stats[:, t, 0:2], bns[:])
                nc.vector.tensor_tensor(
                    stats[:, t, 2:3], stats[:, t, 0:1], stats[:, t, 0:1],
                    op=ALU.mult,
                )
            # group-average (and broadcast) of [mean, var, mean^2] over each
            # 16-partition group: amat is block-diagonal ones/16.
            gps = pmm.tile([P, CT, 3], F32, tag="ps")
            nc.tensor.matmul(gps[:], amat_sb[:], stats[:], start=True, stop=True)
            g = spool.tile([P, CT, 3], F32, tag=f"g{uid}")
            nc.scalar.copy(g[:], gps[:])
            # var_g = E[var] + E[mean^2] - E[mean]^2  (equal-count partitions)
            msq = spool.tile([P, CT], F32, tag=f"msq{uid}")
            nc.vector.tensor_tensor(msq[:], g[:, :, 0], g[:, :, 0], op=ALU.mult)
            var = spool.tile([P, CT], F32, tag=f"var{uid}")
            nc.vector.tensor_tensor(var[:], g[:, :, 1], g[:, :, 2], op=ALU.add)
            nc.vector.tensor_tensor(var[:], var[:], msq[:], op=ALU.subtract)
            nc.vector.tensor_scalar(var[:], var[:], EPS, None, op0=ALU.add)
            # rstd = 1/sqrt(var+eps), Newton-polished
            std = spool.tile([P, CT], F32, tag=f"std{uid}")
            nc.scalar.activation(std[:], var[:], AF.Sqrt)
            r0 = spool.tile([P, CT], F32, tag=f"r0{uid}")
            nc.vector.reciprocal(r0[:], std[:])
            t7 = spool.tile([P, CT], F32, tag=f"t7{uid}")
            nc.vector.tensor_tensor(t7[:], r0[:], r0[:], op=ALU.mult)
            nc.vector.tensor_tensor(t7[:], var[:], t7[:], op=ALU.mult)
            nc.vector.tensor_scalar(t7[:], t7[:], -0.5, 1.5, op0=ALU.mult, op1=ALU.add)
            rstd = spool.tile([P, CT], F32, tag=f"rs{uid}")
            nc.vector.tensor_tensor(rstd[:], r0[:], t7[:], op=ALU.mult)
            # a = rstd*gamma ; mb = beta - mean*a
            a = spool.tile([P, CT], F32, tag=f"a{uid}")
            nc.vector.tensor_tensor(a[:], rstd[:], prm_sb[:, scale_row, :], op=ALU.mult)
            mb = spool.tile([P, CT], F32, tag=f"mb{uid}")
            nc.vector.tensor_tensor(mb[:], g[:, :, 0], a[:], op=ALU.mult)
            nc.vector.tensor_tensor(mb[:], prm_sb[:, bias_row, :], mb[:], op=ALU.subtract)
            return a, mb

        def gn_apply(src_sb, dst_sb, st):
            a, mb = st
            for t in range(CT):
                nc.vector.tensor_scalar(
                    dst_sb[:, t, :], src_sb[:, t, :],
                    a[:, t : t + 1], mb[:, t : t + 1],
                    op0=ALU.mult, op1=ALU.add,
                )

        # ---- prologue: batch-0 inputs + constants first, then weights ----
        amat_sb = cpool.tile([P, P], F32)
        nc.sync.dma_start(amat_sb[:], amat[:])
        prm_sb = cpool.tile([P, 6, CT], F32)
        nc.sync.dma_start(prm_sb[:], prm.rearrange("p (q t) -> p q t", t=CT))
        ones_mat = cpool.tile([P, P], F32R)
        nc.sync.dma_start(ones_mat[:], aps["ones"][:])

        xy = load_xy(0)
        st_x = gn_stats(xy[0], R_GN_SCALE, R_GN_BIAS, uid="x0")
        st_y = gn_stats(xy[1], R_GN1_SCALE, R_GN1_BIAS, uid="y0")

        # weights after inputs: first needed only once GN(x) is applied
        w_sb = {}
        for name, ap in (("wk", wk), ("wv", wv), ("wq", wq), ("wp", wp)):
            t = wpool.tile([P, CT, C], F32R, tag=name)
            nc.sync.dma_start(t[:], ap.rearrange("p (t o) -> p t o", o=C))
            w_sb[name] = t

        for b in range(BPC):
            outv = out[b].rearrange("p (t n) -> p t n", n=HW)
            x_sb, y_sb = xy

            h_sb = hpool.tile([P, CT, HW], F32R, tag="h")
            gn_apply(x_sb, h_sb, st_x)
            yn_sb = ynpool.tile([P, CT, HW], F32R, tag="yn")
            gn_apply(y_sb, yn_sb, st_y)

            # ---- k = Wk h  (k[c_out, i]) ----
            k_sb = kpool.tile([P, CT, HW], F32R, tag="k")
            for mt in range(CT):
                for nh in range(IB):
                    ps = pmm.tile([P, IBS], F32, tag="ps")
                    for kt in range(CT):
                        nc.tensor.matmul(
                            ps[:],
                            w_sb["wk"][:, kt, mt * P : (mt + 1) * P],
                            h_sb[:, kt, nh * IBS : (nh + 1) * IBS],
                            start=(kt == 0), stop=(kt == CT - 1),
                        )
                    nc.scalar.copy(k_sb[:, mt, nh * IBS : (nh + 1) * IBS], ps[:])

            # ---- vT[j, c_out] = h^T WvT ----
            vT_sb = vpool.tile([P, JT, C], F32R, tag="vT")
            for jt in range(JT):
                ps = pmm.tile([P, C], F32, tag="ps")
                for kt in range(CT):
                    nc.tensor.matmul(
                        ps[:],
                        h_sb[:, kt, jt * P : (jt + 1) * P],
                        w_sb["wv"][:, kt, :],
                        start=(kt == 0), stop=(kt == CT - 1),
                    )
                nc.scalar.copy(vT_sb[:, jt, :], ps[:])

            # ---- q = Wq y_ + bq ----
            q_sb = qpool.tile([P, CT, HW], F32R, tag="q")
            for mt in range(CT):
                for nh in range(IB):
                    ps = pmm.tile([P, IBS], F32, tag="ps")
                    for kt in range(CT):
                        nc.tensor.matmul(
                            ps[:],
                            w_sb["wq"][:, kt, mt * P : (mt + 1) * P],
                            yn_sb[:, kt, nh * IBS : (nh + 1) * IBS],
                            start=(kt == 0), stop=(kt == CT - 1),
                        )
                    nc.vector.tensor_scalar(
                        q_sb[:, mt, nh * IBS : (nh + 1) * IBS], ps[:],
                        prm_sb[:, R_BQ, mt : mt + 1], None, op0=ALU.add,
                    )

            # prefetch + pre-stat the next batch while attention runs
            if b + 1 < BPC:
                xy = load_xy(b + 1)
                st_x = gn_stats(xy[0], R_GN_SCALE, R_GN_BIAS, uid=f"x{b+1}")
                st_y = gn_stats(xy[1], R_GN1_SCALE, R_GN1_BIAS, uid=f"y{b+1}")

            # ---- attention, one i-block (512 queries) at a time ----
            for ib in range(IB):
                isl = slice(ib * IBS, (ib + 1) * IBS)
                e_sb = epool.tile([P, JT, IBS], F32R, tag="e")
                zb = pzb.tile([P, IBS], F32, tag="zb")
                for jt in range(JT):
                    ps = pmm.tile([P, IBS], F32, tag="ps")
                    for kt in range(CT):
                        nc.tensor.matmul(
                            ps[:],
                            k_sb[:, kt, jt * P : (jt + 1) * P],
                            q_sb[:, kt, isl],
                            start=(kt == 0), stop=(kt == CT - 1),
                        )
                    # E = exp(S / sqrt(c)); logits are O(1), no max needed
                    nc.scalar.activation(e_sb[:, jt, :], ps[:], AF.Exp, scale=SM_SCALE)
                    # Z[p, i] = sum_j E[j, i] for every p (ones matmul broadcast)
                    nc.tensor.matmul(
                        zb[:], ones_mat[:], e_sb[:, jt, :],
                        start=(jt == 0), stop=(jt == JT - 1),
                    )
                rzb = rzpool.tile([P, IBS], F32, tag="rzb")
                nc.vector.reciprocal(rzb[:], zb[:])

                o0_sb = opool.tile([P, CT, IBS], F32R, tag="o0")
                for ct in range(CT):
                    ps = pmm.tile([P, IBS], F32, tag="ps")
                    for jt in range(JT):
                        nc.tensor.matmul(
                            ps[:],
                            vT_sb[:, jt, ct * P : (ct + 1) * P],
                            e_sb[:, jt, :],
                            start=(jt == 0), stop=(jt == JT - 1),
                        )
                    nc.vector.tensor_tensor(o0_sb[:, ct, :], ps[:], rzb[:], op=ALU.mult)

                # ---- r = Wp o0n + bp' ; out = x + r ----
                for mt in range(CT):
                    ps = pmm.tile([P, IBS], F32, tag="ps")
                    for ct in range(CT):
                        nc.tensor.matmul(
                            ps[:],
                            w_sb["wp"][:, ct, mt * P : (mt + 1) * P],
                            o0_sb[:, ct, :],
                            start=(ct == 0), stop=(ct == CT - 1),
                        )
                    ot = outpool.tile([P, IBS], F32, tag="ot")
                    nc.vector.scalar_tensor_tensor(
                        ot[:], ps[:], prm_sb[:, R_BPP, mt : mt + 1],
                        x_sb[:, mt, isl], op0=ALU.add, op1=ALU.add,
                    )
                    nc.sync.dma_start(outv[:, mt, isl], ot[:])


_CACHE = {}


def _build():
    if "nc" in _CACHE:
        return _CACHE["nc"]
    nc = bacc.Bacc("TRN2", target_bir_lowering=False, debug=False)
    aps = {
        "xs": nc.dram_tensor("xs", [BPC, P, CT * HW], F32, kind="ExternalInput").ap(),
        "ys": nc.dram_tensor("ys", [BPC, P, CT * HW], F32, kind="ExternalInput").ap(),
        "wqT": nc.dram_tensor("wqT", [P, CT * C], F32R, kind="ExternalInput").ap(),
        "wkT": nc.dram_tensor("wkT", [P, CT * C], F32R, kind="ExternalInput").ap(),
        "wvT": nc.dram_tensor("wvT", [P, CT * C], F32R, kind="ExternalInput").ap(),
        "wpT": nc.dram_tensor("wpT", [P, CT * C], F32R, kind="ExternalInput").ap(),
        "prm": nc.dram_tensor("prm", [P, 6 * CT], F32, kind="ExternalInput").ap(),
        "amat": nc.dram_tensor("amat", [P, P], F32, kind="ExternalInput").ap(),
        "ones": nc.dram_tensor("ones", [P, P], F32R, kind="ExternalInput").ap(),
        "out": nc.dram_tensor("out", [BPC, P, CT * HW], F32, kind="ExternalOutput").ap(),
    }
    with tile.TileContext(nc) as tc:
        _emit(tc, aps)
    nc.compile()
    _CACHE["nc"] = nc
    return nc


def _pack_chw(a):
    """[*, C, HW] -> [*, P, CT*HW] matching SBUF layout c = t*128 + p."""
    lead = a.shape[:-2]
    a = a.reshape(*lead, CT, P, HW)
    a = np.moveaxis(a, -3, -2)          # [..., P, CT, HW]
    return np.ascontiguousarray(a.reshape(*lead, P, CT * HW))


def _unpack_chw(a):
    """[*, P, CT*HW] -> [*, C, HW]."""
    lead = a.shape[:-2]
    a = a.reshape(*lead, P, CT, HW)
    a = np.moveaxis(a, -2, -3)          # [..., CT, P, HW]
    return np.ascontiguousarray(a.reshape(*lead, CT * P, HW))


def _round_fp32r(a):
    """Round fp32 to the PE's fp32r format: 1+8+11 bits, low 12 zeroed."""
    u = np.ascontiguousarray(a, dtype=np.float32).view(np.uint32)
    u = u + np.uint32(0x7FF) + ((u >> np.uint32(12)) & np.uint32(1))
    u = u & np.uint32(0xFFFFF000)
    return u.view(np.float32)


def _host_inputs(x, y, norm_scale, norm_bias, norm1_scale, norm1_bias,
                 wq, bq, wk, bk, wv, bv, wp, bp):
    f = lambda a: np.ascontiguousarray(np.asarray(a, dtype=np.float32))
    x = f(x).reshape(B, C, HW)
    y = f(y).reshape(B, C, HW)
    wq, wk, wv, wp = f(wq), f(wk), f(wv), f(wp)
    # bk cancels in softmax; bv folds into bp' because softmax rows sum to 1
    bpp = f(bp) + wp @ f(bv)
    prm = np.stack([f(norm_scale), f(norm_bias), f(norm1_scale), f(norm1_bias),
                    f(bq), bpp]).astype(np.float32)
    # [6, C] -> [P, 6*CT] matching prm_sb[p, q, t]
    prm = np.ascontiguousarray(
        prm.reshape(6, CT, P).transpose(2, 0, 1).reshape(P, 6 * CT)
    )
    amat = np.zeros((P, P), np.float32)
    for g in range(P // GSIZE):
        amat[g * GSIZE : (g + 1) * GSIZE, g * GSIZE : (g + 1) * GSIZE] = 1.0 / GSIZE
    def packw(w):
        # wT [c_in, c_out] -> [P, CT*C] matching w_sb[p, kt, o]
        wT = _round_fp32r(w.T)
        return np.ascontiguousarray(
            wT.reshape(CT, P, C).transpose(1, 0, 2).reshape(P, CT * C)
        )

    shared = {
        "wqT": packw(wq), "wkT": packw(wk), "wvT": packw(wv), "wpT": packw(wp),
        "prm": prm, "amat": amat, "ones": np.ones((P, P), np.float32),
    }
    in_maps = []
    for core in range(NCORES):
        sl = slice(core * BPC, (core + 1) * BPC)
        in_maps.append({
            "xs": _pack_chw(x[sl]),
            "ys": _pack_chw(y[sl]),
            **shared,
        })
    return in_maps


def _run(in_maps, trace=False):
    nc = _build()
    res = run_bass_kernel_spmd(
        nc, in_maps, core_ids=list(range(NCORES)), trace=trace
    )
    out = np.concatenate(
        [_unpack_chw(res.results[i]["out"]) for i in range(NCORES)], axis=0
    ).reshape(B, C, H, W)
    return out, res


def kernel(**inputs):
    in_maps = _host_inputs(**inputs)
    out, _ = _run(in_maps, trace=False)
    return out
